# revision 4
# baseline (speedup 1.0000x reference)
"""Fused single-head attention (projections + softmax attention) on 8 TRN2
NeuronCores.

Problem: B=4, S=4096, H=1024, D=64
  q = query @ Wq + bq ; k = key @ Wk + bk ; v = value @ Wv + bv
  out = softmax(q k^T / sqrt(D), mask over k) @ v

Sharding: core c -> (batch b = c//2, query half h = c%2). Each core
computes 2048 queries against the full 4096 keys/values of its batch.
No collectives.

Layout strategy:
  - Host feeds bf16 transposed shards qT/kT/vT [H, seq] plus bf16
    weights; biases/mask stay f32.
  - Projections col-tiled 2x: the 8 H-chunks are processed as 4
    concurrent pairs on PE col-groups (0,0)/(0,64); psum halves merged
    (+bias) by one DVE scalar_tensor_tensor into bf16 SBUF.
  - k_projT/q_projT stored [128, seq] with rows 64:128 duplicating
    rows 0:64 so score matmuls can row-tile: K=64 stationaries at
    row-groups (0,0)/(64,0) run two key-tiles concurrently.
  - Scores transposed: sT[k, q]; groups of 3 key tiles (pair + solo)
    share one [128, 3, 512] psum tile; exp activation N=1536 amortizes
    the 352-cycle ACT overhead. No max-subtraction: |s|/8 <~ 3.
  - v PE-transposed tile-wise into v_aug [k, 65] bf16 with the mask
    folded in: v_aug = [v*m | m]; row 64 of att psum = softmax denom.
  - att@v: stationary v_aug[t], moving expT, N=512, accumulated over
    32 key tiles per 512-query chunk.
  - Scalar engine does (almost) nothing but exp: table pre-loaded via a
    dummy activation at t~0; 14 junk warm-up matmuls promote the PE HAM
    clock gate before real data lands.
"""

import ml_dtypes
import numpy as np

import concourse.bass as bass
import concourse.mybir as mybir
import concourse.tile as tile
from concourse.masks import make_identity
from concourse.vector_clock import ScopedClock

B, S, H, D = 4, 4096, 1024, 64
NCORES = 8
SQ = S // 2          # queries per core
HT = H // 128        # 8 contraction chunks
TK = S // 128        # 32 key tiles
QCH = 512            # matmul moving free dim
NQC = SQ // QCH      # 4 query chunks per core

FP = mybir.dt.float32
BF = mybir.dt.bfloat16

# ---------------------------------------------------------------------------
# Walrus in this container rejects >1 sync-wait per instruction; peel extra
# waits onto same-engine nops (engine streams are in-order).
_orig_commit = tile.TileContext._commit_instruction


def _split_waits(self, inst):
    si = inst.sync_info
    if si is None or not si.on_wait or len(si.on_wait) <= 1:
        return
    waits = list(si.on_wait)
    si.on_wait = waits[-1:]
    for w in waits[:-1]:
        nop = mybir.InstNoOp(
            name=self.nc.get_next_instruction_name(),
            sync_info=mybir.SyncInfo(on_wait=[w], on_update=[]),
            bass_nofuse=True,
            engine=inst.engine,
            ins=[],
            outs=[],
        )
        _orig_commit(self, nop)


def _patched_commit(self, inst, lazy_reg_writes=True):
    _split_waits(self, inst)
    return _orig_commit(self, inst, lazy_reg_writes)


def _patched_drain_and_barrier(self, tick_clock, wait_clock):
    nc = self.nc
    collector = nc.sync.nop(nofuse=True, hint="tile_drain_waits")
    wait_clock.add_sem_waits(
        collector.ins, ScopedClock({None: tick_clock.global_clock})
    )
    si = collector.ins.sync_info
    if si is not None and si.on_wait and len(si.on_wait) > 1:
        waits = list(si.on_wait)
        si.on_wait = waits[:1]
        for w in waits[1:]:
            extra = nc.sync.nop(nofuse=True, hint="tile_drain_waits")
            if extra.ins.sync_info is None:
                extra.ins.sync_info = mybir.SyncInfo(on_wait=[w], on_update=[])
            else:
                extra.ins.sync_info.on_wait = [w]
    nc.sync.drain()
    nc.all_engine_barrier()
    assert self.sems is not None
    popped = nc._tile_sem_poison_stack.pop()
    assert popped is self._sem_poison
    nc.clear_and_free_semaphores(list(self.sems.allocated().values()))
    nc.all_engine_barrier()


tile.TileContext._commit_instruction = _patched_commit
tile.TileContext._drain_and_barrier = _patched_drain_and_barrier
# ---------------------------------------------------------------------------

AF = mybir.ActivationFunctionType
ALU = mybir.AluOpType


def _build():
    nc = bass.Bass(trn_type="TRN2")

    qT = nc.declare_dram_parameter("qT", [H, SQ], BF, isOutput=False)
    kT = nc.declare_dram_parameter("kT", [H, S], BF, isOutput=False)
    vT = nc.declare_dram_parameter("vT", [H, S], BF, isOutput=False)
    maskT = nc.declare_dram_parameter("maskT", [128, TK], FP, isOutput=False)
    wq = nc.declare_dram_parameter("wq", [H, D], BF, isOutput=False)
    wk = nc.declare_dram_parameter("wk", [H, D], BF, isOutput=False)
    wv = nc.declare_dram_parameter("wv", [H, D], BF, isOutput=False)
    bq = nc.declare_dram_parameter("bq", [D, 1], FP, isOutput=False)
    bk = nc.declare_dram_parameter("bk", [D, 1], FP, isOutput=False)
    bv = nc.declare_dram_parameter("bv", [D, 1], FP, isOutput=False)
    outT = nc.declare_dram_parameter("outT", [D, SQ], FP, isOutput=True)

    qT_ap = qT[:, :].rearrange("(o p) s -> p o s", p=128)
    kT_ap = kT[:, :].rearrange("(o p) s -> p o s", p=128)
    vT_ap = vT[:, :].rearrange("(o p) s -> p o s", p=128)
    wq_ap = wq[:, :].rearrange("(o p) d -> p o d", p=128)
    wk_ap = wk[:, :].rearrange("(o p) d -> p o d", p=128)
    wv_ap = wv[:, :].rearrange("(o p) d -> p o d", p=128)

    with tile.TileContext(nc) as tc:
        with (
            tc.tile_pool(name="const", bufs=1) as cpool,
            tc.tile_pool(name="proj", bufs=1) as projpool,
            tc.tile_pool(name="xin", bufs=3) as xpool,
            tc.tile_pool(name="expb", bufs=3) as exppool,
            tc.tile_pool(name="outs", bufs=1) as outpool,
            tc.tile_pool(name="ps", bufs=1, space="PSUM") as pspool,
        ):
            # ---- constants ------------------------------------------------
            warm = cpool.tile([128, 640], BF, tag="warm")
            nc.vector.memset(warm[:], 0.0)
            dume = cpool.tile([128, 8], FP, tag="dume")
            dumo = cpool.tile([128, 8], BF, tag="dumo")
            nc.vector.memset(dume[:], 0.0)

            wq_s = cpool.tile([128, HT, D], BF, tag="wq")
            wk_s = cpool.tile([128, HT, D], BF, tag="wk")
            wv_s = cpool.tile([128, HT, D], BF, tag="wv")
            nc.scalar.dma_start(wk_s[:], wk_ap)
            nc.sync.dma_start(wq_s[:], wq_ap)
            nc.gpsimd.dma_start(wv_s[:], wv_ap)
            bq_s = cpool.tile([D, 1], FP, tag="bq")
            bk_s = cpool.tile([D, 1], FP, tag="bk")
            bv_s = cpool.tile([D, 1], FP, tag="bv")
            nc.sync.dma_start(bq_s[:], bq[:, :])
            nc.sync.dma_start(bk_s[:], bk[:, :])
            nc.sync.dma_start(bv_s[:], bv[:, :])
            maskT_s = cpool.tile([128, TK], FP, tag="mask")
            nc.sync.dma_start(maskT_s[:], maskT[:, :])
            ones64 = cpool.tile([1, D], FP, tag="ones")
            nc.vector.memset(ones64[:], 1.0)

            # exp table pre-load: dummy activation, first thing Scalar does
            nc.scalar.activation(dumo[:], dume[:], AF.Exp, scale=0.125)

            # ---- PE warm-up: junk matmuls promote the HAM clock gate ------
            warm_ps = pspool.tile([128, QCH], FP, tag="att", name="warmps")

            def warm_mms(n):
                for _ in range(n):
                    nc.tensor.matmul(
                        warm_ps[:, :],
                        warm[:, 0:128],
                        warm[:, 128:640],
                        start=True,
                        stop=True,
                    )

            warm_mms(8)

            # ---- projections ---------------------------------------------
            # q_projT/k_projT: [128, seq] with rows 64:128 = copy of 0:64
            # (row-tiled score matmuls need stationaries/moving on both
            # row-groups). v_projT: [64, seq].
            q_projT = projpool.tile([128, SQ], BF, tag="qproj")
            k_projT = projpool.tile([128, S], BF, tag="kproj")
            v_projT = projpool.tile([D, S], BF, tag="vproj")

            CW = 2 * QCH  # 1024-col input chunks

            def proj_chunk(nm, dst, src_ap, w_s, b_s, c0, dup, fill=None):
                xt = xpool.tile(
                    [128, HT, CW], BF, tag="xin", name=f"x{nm}{c0}"
                )
                c1 = c0 + CW
                nc.gpsimd.dma_start(xt[:, 0:3, :], src_ap[:, 0:3, c0:c1])
                nc.scalar.dma_start(xt[:, 3:6, :], src_ap[:, 3:6, c0:c1])
                nc.sync.dma_start(xt[:, 6:8, :], src_ap[:, 6:8, c0:c1])
                for j in range(2):
                    sl = slice(j * QCH, (j + 1) * QCH)
                    csl = slice(c0 + j * QCH, c0 + (j + 1) * QCH)
                    pj = pspool.tile(
                        [128, QCH], FP, tag="tmp", name=f"pj{nm}{c0}_{j}"
                    )
                    for r in range(4):
                        nc.tensor.matmul(
                            pj[0:64, :],
                            w_s[:, r, :],
                            xt[:, r, sl],
                            start=(r == 0),
                            stop=(r == 3),
                            tile_position=(0, 0),
                        )
                        nc.tensor.matmul(
                            pj[64:128, :],
                            w_s[:, 4 + r, :],
                            xt[:, 4 + r, sl],
                            start=(r == 0),
                            stop=(r == 3),
                            tile_position=(0, 64),
                        )
                    if fill is not None:
                        fill()  # keep PE busy while DVE merges
                    # DVE can read only one PSUM operand per instruction:
                    # stage hi half (+bias) to SBUF, then add lo half.
                    hi_s = outpool.tile(
                        [64, QCH], BF, tag="histage",
                        name=f"hi{nm}{c0}_{j}", bufs=2,
                    )
                    nc.vector.tensor_scalar_add(hi_s[:], pj[64:128, :], b_s[:, :])
                    nc.vector.tensor_tensor(
                        dst[0:64, csl], pj[0:64, :], hi_s[:], ALU.add
                    )
                    if dup:
                        nc.vector.tensor_copy(dst[64:128, csl], dst[0:64, csl])

            def k_chunk(i, fill=None):
                proj_chunk("k", k_projT, kT_ap, wk_s, bk_s, i * CW, True, fill)

            def q_chunk(i, fill=None):
                proj_chunk("q", q_projT, qT_ap, wq_s, bq_s, i * CW, True, fill)

            def v_chunk(i, fill=None):
                proj_chunk("v", v_projT, vT_ap, wv_s, bv_s, i * CW, False, fill)

            ident = cpool.tile([D, D], BF, tag="ident")
            make_identity(nc, ident[:])

            # ---- v_aug [128, TK, 128] bf16 = [v*m | m | 0] -----------------
            v_aug = projpool.tile([128, TK, 128], BF, tag="vaug")
            nc.vector.memset(v_aug[:, :, D + 1 :], 0.0)
            nc.vector.tensor_copy(v_aug[:, :, D : D + 1], maskT_s[:, :])

            def v_trans(tp_):  # transposes key tiles tp_, tp_+1
                tp = pspool.tile([128, 2 * QCH], BF, tag="tmp", name=f"tp{tp_}")
                for i in range(2):
                    t = tp_ + i
                    nc.tensor.transpose(
                        tp[:, i * QCH : i * QCH + D],
                        v_projT[:, t * 128 : (t + 1) * 128],
                        ident[:, :],
                    )
                for i in range(2):
                    t = tp_ + i
                    nc.vector.tensor_scalar_mul(
                        v_aug[:, t, :D],
                        tp[:, i * QCH : i * QCH + D],
                        maskT_s[:, t : t + 1],
                    )

            # ---- attention ------------------------------------------------
            outT_s = outpool.tile([D, SQ], FP, tag="outT")
            exp_tiles = {}

            def kd(h, t):
                return k_projT[64 * h : 64 * h + 64, t * 128 : (t + 1) * 128]

            def qd(h, c):
                return q_projT[64 * h : 64 * h + 64, c * QCH : (c + 1) * QCH]

            # scores group g of chunk c: key tiles 3g..3g+2 (pair + solo),
            # one [128, 3*512] psum tile, one exp activation N=1536.
            # g == 10 is the final pair (tiles 30, 31), N=1024.
            def scores_group(c, g):
                if c not in exp_tiles:
                    exp_tiles[c] = exppool.tile(
                        [128, TK, QCH], BF, tag="expT", name=f"expT{c}"
                    )
                expTc = exp_tiles[c]
                t0 = 3 * g
                ntile = 3 if g < 10 else 2
                sp = pspool.tile(
                    [128, 3, QCH], FP, tag="sc", name=f"sp{c}_{g}"
                )
                nc.tensor.matmul(
                    sp[:, 0, :], kd(0, t0), qd(0, c),
                    start=True, stop=True, tile_position=(0, 0),
                )
                nc.tensor.matmul(
                    sp[:, 1, :], kd(1, t0 + 1), qd(1, c),
                    start=True, stop=True, tile_position=(64, 0),
                )
                if ntile == 3:
                    nc.tensor.matmul(
                        sp[:, 2, :], kd(0, t0 + 2), qd(0, c),
                        start=True, stop=True, tile_position=(0, 0),
                    )
                nc.scalar.activation(
                    expTc[:, t0 : t0 + ntile, :],
                    sp[:, 0:ntile, :],
                    AF.Exp,
                    scale=0.125,
                )

            att_ps = {}

            def attv_part(c, lo, hi):
                if c not in att_ps:
                    att_ps[c] = pspool.tile(
                        [128, QCH], FP, tag="att", name=f"att{c}"
                    )
                ap = att_ps[c]
                expTc = exp_tiles[c]
                for t in range(lo, hi):
                    nc.tensor.matmul(
                        ap[:, :],
                        v_aug[:, t, :],
                        expTc[:, t, :],
                        start=(t == 0),
                        stop=(t == TK - 1),
                    )

            def attv_fin(c):
                ap = att_ps.pop(c)
                exp_tiles.pop(c)
                recip = outpool.tile(
                    [1, QCH], FP, tag="recip", name=f"recip{c}", bufs=2
                )
                nc.vector.reciprocal(recip[:], ap[D : D + 1, :])
                rb = pspool.tile([128, QCH], FP, tag="tmp", name=f"rb{c}")
                nc.tensor.matmul(
                    rb[:D, :], ones64[:, :], recip[:, :], start=True, stop=True
                )
                rbs = outpool.tile(
                    [D, QCH], FP, tag="rbs", name=f"rbs{c}", bufs=2
                )
                nc.vector.tensor_copy(rbs[:], rb[:D, :])
                nc.vector.tensor_tensor(
                    outT_s[:, c * QCH : (c + 1) * QCH],
                    ap[:D, :],
                    rbs[:],
                    ALU.mult,
                )
                eng = nc.gpsimd if c % 2 == 0 else nc.sync
                eng.dma_start(
                    outT[:, c * QCH : (c + 1) * QCH],
                    outT_s[:, c * QCH : (c + 1) * QCH],
                )

            # ---- emission order (PE program order == execution order) -----
            # DMA need-order: k0 q0 k1 k2 k3 v0 q1 v1 v2 v3
            k_chunk(0, fill=lambda: warm_mms(3))   # k tiles 0-7
            q_chunk(0, fill=lambda: warm_mms(3))   # q cols 0:1024
            scores_group(0, 0)
            scores_group(0, 1)
            k_chunk(1)                              # k tiles 8-15
            for g in range(2, 5):
                scores_group(0, g)
            k_chunk(2)                              # k tiles 16-23
            for g in range(5, 8):
                scores_group(0, g)
            k_chunk(3)                              # k tiles 24-31
            for g in range(8, 11):
                scores_group(0, g)
            scores_group(1, 0)
            scores_group(1, 1)
            v_chunk(0)                              # v tiles 0-7
            for t in range(0, 8, 2):
                v_trans(t)
            for g in range(2, 6):
                scores_group(1, g)
            v_chunk(1)                              # v tiles 8-15
            for t in range(8, 16, 2):
                v_trans(t)
            for g in range(6, 11):
                scores_group(1, g)
            attv_part(0, 0, 16)
            q_chunk(1)                              # q cols 1024:2048
            scores_group(2, 0)
            scores_group(2, 1)
            scores_group(2, 2)
            v_chunk(2)                              # v tiles 16-23
            for t in range(16, 24, 2):
                v_trans(t)
            for g in range(3, 7):
                scores_group(2, g)
            attv_part(0, 16, 24)
            v_chunk(3)                              # v tiles 24-31
            for t in range(24, 32, 2):
                v_trans(t)
            for g in range(7, 11):
                scores_group(2, g)
            attv_part(0, 24, 32)
            attv_fin(0)
            for g in range(0, 4):
                scores_group(3, g)
            attv_part(1, 0, 16)
            for g in range(4, 8):
                scores_group(3, g)
            attv_part(1, 16, 32)
            attv_fin(1)
            for g in range(8, 11):
                scores_group(3, g)
            attv_part(2, 0, 32)
            attv_fin(2)
            attv_part(3, 0, 24)
            attv_part(3, 24, 32)
            attv_fin(3)

    return nc


_NC_CACHE = None
LAST_RESULT = None


def kernel(query, key, value, mask, Wq, bq, Wk, bk, Wv, bv):
    global _NC_CACHE, LAST_RESULT
    from concourse.bass_utils import run_bass_kernel_spmd

    bf16 = ml_dtypes.bfloat16
    query = np.asarray(query, np.float32)
    key = np.asarray(key, np.float32)
    value = np.asarray(value, np.float32)
    maskf = np.asarray(mask).astype(np.float32)
    Wqb = np.asarray(Wq, np.float32).astype(bf16)
    Wkb = np.asarray(Wk, np.float32).astype(bf16)
    Wvb = np.asarray(Wv, np.float32).astype(bf16)
    bq = np.asarray(bq, np.float32).reshape(D, 1)
    bk = np.asarray(bk, np.float32).reshape(D, 1)
    bv = np.asarray(bv, np.float32).reshape(D, 1)

    in_maps = []
    for c in range(NCORES):
        b, h = divmod(c, 2)
        qs = slice(h * SQ, (h + 1) * SQ)
        in_maps.append(
            {
                "qT": np.ascontiguousarray(query[b, qs].T).astype(bf16),
                "kT": np.ascontiguousarray(key[b].T).astype(bf16),
                "vT": np.ascontiguousarray(value[b].T).astype(bf16),
                "maskT": np.ascontiguousarray(maskf[b].reshape(TK, 128).T),
                "wq": Wqb,
                "wk": Wkb,
                "wv": Wvb,
                "bq": bq,
                "bk": bk,
                "bv": bv,
            }
        )

    if _NC_CACHE is None:
        _NC_CACHE = _build()

    res = run_bass_kernel_spmd(
        _NC_CACHE, in_maps, core_ids=list(range(NCORES))
    )
    LAST_RESULT = res

    outv = np.empty((B, S, D), np.float32)
    for c in range(NCORES):
        b, h = divmod(c, 2)
        outv[b, h * SQ : (h + 1) * SQ] = res.results[c]["outT"].T
    return outv
